# revision 33
# baseline (speedup 1.0000x reference)
"""Trainium2 Bass kernel: causal multi-head attention with RoPE (B=1, S=4096,
D=768, H=12) distributed over 8 NeuronCores.

Sharding strategy
-----------------
- Q rows are strided across cores (core c owns rows r = c mod 8) so causal
  work is uniform across cores (the SPMD program is identical on every core).
- K/V projections are computed on contiguous 512-row shards per core, RoPE'd
  and transposed locally, then AllGather'd so every core holds full K/V.
- Attention runs in "transposed scores" layout: S^T[k, q] = K_rope @ Q_rope^T
  so the AV matmul consumes exp(S^T) directly, and a ones-column appended to V
  yields the softmax denominators in the same accumulation.  Softmax is
  computed without max-subtraction (scores ~N(0,1)).
- All math is bf16 (fp8 q/k quantization alone costs 2.7e-2 relative error —
  over the accuracy gate — so the tensor engine runs bf16 throughout).
- AV matmuls are software-pipelined several batches behind their exp so the
  in-order PE stream never stalls on the later-arriving gathered V.
- RoPE pairs are de-interleaved by permuting W_q/W_k columns host-side so the
  rotation is a full-width unit-stride vector op.
- All DRAM inputs are partition-major contiguous so every load is one
  descriptor per partition.
"""

import math
import os as _os
import sys

import numpy as np

sys.path.insert(0, "/opt/trn_rl_repo")

import ml_dtypes

import concourse.bass as bass
import concourse.mybir as mybir
import concourse.tile as tile
from concourse import bacc
from concourse.masks import make_identity

BF = ml_dtypes.bfloat16
F32 = mybir.dt.float32
BF16 = mybir.dt.bfloat16

S, D, H, DH = 4096, 768, 12, 64
NC = 8
SL = S // NC          # 512 rows per core (both q-strided and kv-contiguous)
NJ = SL // 128        # 4 row-tiles per core
NM = S // 128         # 32 k-tiles
NDC = D // 128        # 6 contraction chunks == head pairs
H32 = H * 32          # 384

# Concurrent xbar transposes on two HWDGE queues race on real hardware
# (verified: nondeterministic corruption) — keep them on one queue.
F_T2Q = _os.environ.get("K_T2Q", "0") == "1"
F_LAG = int(_os.environ.get("K_LAG", "9"))       # AV software-pipeline depth
F_WARM = _os.environ.get("K_WARM", "1") == "1"   # PE p-state warmup
F_BC0 = _os.environ.get("K_BC0", "1") == "1"     # stride-0 cos/sin broadcast
F_PET = _os.environ.get("K_PET", "1") == "1"     # PE transposes + ACT copies
# DVE Schraudolph exp (int16 bitcast) relieves the scalar engine but puts
# the slower DVE inside the score-PSUM recycle chain: measured +4.5us. Off.
F_SCH = _os.environ.get("K_SCH", "0") == "1"


def build_nc():
    nc = bacc.Bacc(None, target_bir_lowering=False, debug=False)

    xq_t = nc.dram_tensor("xq_t", [128, NDC * SL], BF16, kind="ExternalInput")
    xkv_t = nc.dram_tensor("xkv_t", [128, NDC * SL], BF16, kind="ExternalInput")
    wq = nc.dram_tensor("wq", [128, NDC * D], BF16, kind="ExternalInput")
    wk = nc.dram_tensor("wk", [128, NDC * D], BF16, kind="ExternalInput")
    wv = nc.dram_tensor("wv", [128, NDC * D], BF16, kind="ExternalInput")
    wo = nc.dram_tensor("wo", [128, NDC * D], BF16, kind="ExternalInput")
    cosq = nc.dram_tensor("cosq", [128, NJ * 32], BF16, kind="ExternalInput")
    sinq = nc.dram_tensor("sinq", [128, NJ * 32], BF16, kind="ExternalInput")
    cosk = nc.dram_tensor("cosk", [128, NJ * 32], BF16, kind="ExternalInput")
    sink = nc.dram_tensor("sink", [128, NJ * 32], BF16, kind="ExternalInput")
    mask8 = nc.dram_tensor("mask8", [128, 8 * 128], BF16, kind="ExternalInput")
    y_d = nc.dram_tensor("y", [SL, D], BF16, kind="ExternalOutput")

    KT_N = 128 * NDC * SL             # elements of one core's k^T shard
    V_N = 128 * NJ * H * (DH + 1)

    with tile.TileContext(nc) as tc:
        # ---- persistent pool (lives to the end) ----
        P1 = tc.alloc_tile_pool(name="persist", bufs=1)
        wo_sb = P1.tile([128, NDC, D], BF16)
        mk_sb = P1.tile([128, 8, 128], BF16)
        qt_sb = P1.tile([128, NDC, SL], BF16)         # q^T (rope'd)
        att_sb = P1.tile([128, NDC, SL], BF16)        # attention out^T (normed)
        ktg = P1.tile([128, NC, NDC, SL], BF16)       # gathered k^T, r-outer
        vog = P1.tile([128, NC, NJ, H, DH + 1], BF16)  # gathered V (+ones col)

        PD = tc.alloc_tile_pool(name="dram", bufs=1, space="DRAM")
        kt_b = PD.tile([KT_N], BF16)
        v_b = PD.tile([V_N], BF16)
        kt_g = PD.tile([NC * KT_N], BF16, addr_space="Shared")
        v_g = PD.tile([NC * V_N], BF16, addr_space="Shared")

        # ---- projection + rope + transpose for one stream ----
        # r_sb column order per head: [y0(32) | y1(32)], heads in order, so
        # the per-(st, dc) [128,128] transpose lands chunk dc's two heads on
        # partitions [0:64) / [64:128) — the K=64 score-matmul layout.
        def proj_rope_t(x_sb, w_sb, cos_sb, sin_sb, dst_bf, ps_bufs=2,
                        warm=None, ident=None):
            PP = tc.alloc_tile_pool(name="proj_ps", bufs=ps_bufs, space="PSUM")
            if F_PET:
                PT = tc.alloc_tile_pool(name="tr_ps", bufs=3, space="PSUM")
            PW = tc.alloc_tile_pool(name="proj_work", bufs=2)
            if warm is not None and F_WARM:
                w_ps = PP.tile([128, 512], F32, tag="warm", bufs=1)
                for _ in range(9):
                    nc.tensor.matmul(w_ps, warm[:, 0:128], warm,
                                     start=True, stop=True)
            pend_t = []
            for st in range(NJ):
                n_ps = PP.tile([128, D], F32, tag="n_ps")
                for dc in range(NDC):
                    lt = x_sb[:, dc, st * 128:(st + 1) * 128]
                    nc.tensor.matmul(n_ps[:, 0:512], lt, w_sb[:, dc, 0:512],
                                     start=(dc == 0), stop=(dc == NDC - 1))
                    nc.tensor.matmul(n_ps[:, 512:768], lt, w_sb[:, dc, 512:768],
                                     start=(dc == 0), stop=(dc == NDC - 1))
                # previous row-tile's transposes go to the PE *after* this
                # tile's matmuls so the in-order PE never waits on the rope
                for fn_ in pend_t:
                    fn_()
                pend_t = []
                nb = PW.tile([128, H, 2, 32], BF16, tag="nb")
                nc.vector.tensor_copy(
                    nb.rearrange("p h x i -> p (h x i)"), n_ps)
                x0 = nb[:, :, 0]
                x1 = nb[:, :, 1]
                c0 = cos_sb[:, st]
                s0 = sin_sb[:, st]
                if F_BC0:
                    cs = bass.AP(tensor=c0.tensor, offset=c0.offset,
                                 ap=[list(c0.ap[0]), [0, H], [1, 32]])
                    sn = bass.AP(tensor=s0.tensor, offset=s0.offset,
                                 ap=[list(s0.ap[0]), [0, H], [1, 32]])
                else:
                    csf = PW.tile([128, H, 32], BF16, tag="csf")
                    snf = PW.tile([128, H, 32], BF16, tag="snf")
                    for h in range(H):
                        nc.vector.tensor_copy(csf[:, h], c0)
                        nc.vector.tensor_copy(snf[:, h], s0)
                    cs, sn = csf, snf
                ta = PW.tile([128, H, 32], BF16, tag="ta")
                tb = PW.tile([128, H, 32], BF16, tag="tb")
                tc2 = PW.tile([128, H, 32], BF16, tag="tc")
                td = PW.tile([128, H, 32], BF16, tag="td")
                r_sb = PW.tile([128, H, 2, 32], BF16, tag="r_sb")
                nc.vector.tensor_mul(ta, x0, cs)
                nc.vector.tensor_mul(tb, x1, sn)
                nc.vector.tensor_sub(r_sb[:, :, 0], ta, tb)
                nc.vector.tensor_mul(tc2, x0, sn)
                nc.vector.tensor_mul(td, x1, cs)
                nc.vector.tensor_add(r_sb[:, :, 1], tc2, td)
                rf = r_sb.rearrange("p h x i -> p (h x i)")
                if F_PET:
                    def tjob(rf=rf, st=st):
                        # PE transpose + copy on the (idle) scalar engine
                        for dc in range(NDC):
                            t_ps = PT.tile([128, 128], BF16, tag="t_ps")
                            nc.tensor.transpose(
                                t_ps, rf[:, dc * 128:(dc + 1) * 128], ident)
                            nc.scalar.activation(
                                dst_bf[:, dc, st * 128:(st + 1) * 128], t_ps,
                                mybir.ActivationFunctionType.Copy)
                    pend_t.append(tjob)
                else:
                    for dc in range(NDC):
                        eng = (nc.sync if dc % 2 == 0 or not F_T2Q
                               else nc.scalar)
                        eng.dma_start(
                            out=dst_bf[:, dc, st * 128:(st + 1) * 128],
                            in_=rf[:, dc * 128:(dc + 1) * 128],
                            transpose=True)
            for fn_ in pend_t:
                fn_()
            PW.release()
            if F_PET:
                PT.release()
            PP.release()

        def v_proj(x_sb, v_w_sb, v_dst):
            PP = tc.alloc_tile_pool(name="vproj_ps", bufs=2, space="PSUM")
            for st in range(NJ):
                v_ps = PP.tile([128, D], F32, tag="v_ps")
                for dc in range(NDC):
                    lt = x_sb[:, dc, st * 128:(st + 1) * 128]
                    nc.tensor.matmul(v_ps[:, 0:512], lt, v_w_sb[:, dc, 0:512],
                                     start=(dc == 0), stop=(dc == NDC - 1))
                    nc.tensor.matmul(v_ps[:, 512:768], lt,
                                     v_w_sb[:, dc, 512:768],
                                     start=(dc == 0), stop=(dc == NDC - 1))
                nc.vector.tensor_copy(
                    v_dst[:, st, :, 0:DH],
                    v_ps.rearrange("p (h d) -> p h d", h=H))
            PP.release()

        # ---- input loads (K-path inputs first; Q/O loads deferred) ----
        P2 = tc.alloc_tile_pool(name="kv_in", bufs=1)
        wk_sb = P2.tile([128, NDC, D], BF16)
        xkv_sb = P2.tile([128, NDC, SL], BF16)
        HC, HD, HS = NDC // 2, NDC // 2 * D, NDC // 2 * SL
        nc.sync.dma_start(out=wk_sb[:, 0:HC].rearrange("p c d -> p (c d)"),
                          in_=wk[:, 0:HD])
        nc.sync.dma_start(out=xkv_sb[:, 0:HC].rearrange("p c s -> p (c s)"),
                          in_=xkv_t[:, 0:HS])
        nc.sync.dma_start(out=wk_sb[:, HC:].rearrange("p c d -> p (c d)"),
                          in_=wk[:, HD:])
        nc.sync.dma_start(out=xkv_sb[:, HC:].rearrange("p c s -> p (c s)"),
                          in_=xkv_t[:, HS:])
        ck_sb = P2.tile([128, NJ, 32], BF16)
        nc.scalar.dma_start(out=ck_sb.rearrange("p t d -> p (t d)"), in_=cosk[:, :])
        sk_sb = P2.tile([128, NJ, 32], BF16)
        nc.scalar.dma_start(out=sk_sb.rearrange("p t d -> p (t d)"), in_=sink[:, :])
        P3 = tc.alloc_tile_pool(name="q_in", bufs=1)
        cq_sb = P3.tile([128, NJ, 32], BF16)
        nc.scalar.dma_start(out=cq_sb.rearrange("p t d -> p (t d)"), in_=cosq[:, :])
        sq_sb = P3.tile([128, NJ, 32], BF16)
        nc.scalar.dma_start(out=sq_sb.rearrange("p t d -> p (t d)"), in_=sinq[:, :])
        wv_sb = P2.tile([128, NDC, D], BF16)
        nc.sync.dma_start(out=wv_sb.rearrange("p c d -> p (c d)"), in_=wv[:, :])
        wq_sb = P3.tile([128, NDC, D], BF16)
        xq_sb = P3.tile([128, NDC, SL], BF16)
        kts_bf = P2.tile([128, NDC, SL], BF16)
        vs_sb = P2.tile([128, NJ, H, DH + 1], BF16)
        nc.vector.memset(vs_sb[:, :, :, DH:DH + 1], 1.0)
        warm_sb = P2.tile([128, 512], BF16)
        nc.vector.memset(warm_sb, 0.0)
        ident = P2.tile([128, 128], BF16)
        make_identity(nc, ident)

        # ---- K shard (critical path to the AllGather) ----
        proj_rope_t(xkv_sb, wk_sb, ck_sb, sk_sb, kts_bf, warm=warm_sb,
                    ident=ident)
        nc.sync.dma_start(
            out=kt_b[:].rearrange("(p n) -> p n", p=128),
            in_=kts_bf.rearrange("p c s -> p (c s)"))
        nc.gpsimd.collective_compute(
            "AllGather", mybir.AluOpType.bypass,
            replica_groups=[list(range(NC))],
            ins=[kt_b[:]], outs=[kt_g[:]],
        )
        # deferred loads: issued only after the K-path DMAs so they don't
        # crowd the descriptor channel ahead of the first collective
        nc.scalar.dma_start(out=wq_sb.rearrange("p c d -> p (c d)"), in_=wq[:, :])
        nc.scalar.dma_start(out=xq_sb.rearrange("p c s -> p (c s)"), in_=xq_t[:, :])
        nc.scalar.dma_start(out=wo_sb.rearrange("p c d -> p (c d)"), in_=wo[:, :])
        nc.scalar.dma_start(
            out=mk_sb.rearrange("p m q -> p (m q)"), in_=mask8[:, :])

        # ---- V shard (store must land before the V AllGather slot) ----
        v_proj(xkv_sb, wv_sb, vs_sb)
        nc.sync.dma_start(
            out=v_b[:].rearrange("(p n) -> p n", p=128),
            in_=vs_sb.rearrange("p t h e -> p (t h e)"))
        nc.gpsimd.collective_compute(
            "AllGather", mybir.AluOpType.bypass,
            replica_groups=[list(range(NC))],
            ins=[v_b[:]], outs=[v_g[:]],
        )

        # ---- Q shard (overlaps the collectives) ----
        proj_rope_t(xq_sb, wq_sb, cq_sb, sq_sb, qt_sb, ident=ident)
        P3.release()
        P2.release()
        PS = tc.alloc_tile_pool(name="sc_ps", bufs=2, space="PSUM")

        # ---- load gathered K/V into SBUF caches ----
        ktg_view = kt_g.rearrange("(r p c s) -> r p c s", r=NC, p=128, c=NDC)
        nc.sync.dma_start(out=ktg[:, 0, 0], in_=ktg_view[0, :, 0])
        nc.sync.dma_start(
            out=ktg[:, 0, 1:].rearrange("p c s -> p (c s)"),
            in_=ktg_view[0, :, 1:].rearrange("p c s -> p (c s)"))
        for r in range(1, NC):
            nc.sync.dma_start(
                out=ktg[:, r].rearrange("p c s -> p (c s)"),
                in_=ktg_view[r].rearrange("p c s -> p (c s)"))
        vg_view = v_g.rearrange("(r p n) -> r p n", r=NC, p=128)
        for r in range(NC):
            nc.gpsimd.dma_start(
                out=vog[:, r].rearrange("p t h e -> p (t h e)"),
                in_=vg_view[r])

        # ---- attention ----
        PO = tc.alloc_tile_pool(name="o_ps", bufs=2, space="PSUM")
        PA = tc.alloc_tile_pool(name="att_work", bufs=22)
        PB = tc.alloc_tile_pool(name="bc_work", bufs=2)
        PRD = tc.alloc_tile_pool(name="rd_dram", bufs=2, space="DRAM")

        # AV matmuls are emitted LAG batches behind their exp so the in-order
        # PE stream never stalls on the (later-arriving) gathered V.
        LAG = F_LAG
        pend = []          # (emit_av_closure, normalize_closure_or_None)

        def flush(n):
            while len(pend) > n:
                av, fin = pend.pop(0)
                av()
                if fin is not None:
                    fin()

        for hp in range(H // 2):
            o_A = PO.tile([DH + 1, SL], F32, tag="o_ps", name="o_A")
            o_B = PO.tile([DH + 1, SL], F32, tag="o_ps", name="o_B")
            for g in range(NJ):
                qoff = g * 128
                ml0 = 0
                while ml0 < 8:
                    woff = qoff + 16 * ml0
                    wb = SL - woff
                    # slot stride: each member must stay inside one PSUM bank
                    slot = 512 if wb > 256 else (256 if wb > 128 else 128)
                    nb = min(8 - ml0, 1536 // slot)
                    mw = 128 - 16 * ml0
                    sc_A = PS.tile([128, 1536], F32, tag="sc_ps", name="sc_A")
                    sc_B = PS.tile([128, 1536], F32, tag="sc_ps", name="sc_B")
                    # head pairs 2-3 route the odd head's exp to the vector
                    # engine: bf16 Schraudolph (int16(x*A+B) bitcast to bf16)
                    # relieves the scalar engine, and the bitcast view keeps
                    # the mask multiply in 4x mode and feeds AV directly.
                    offl = F_SCH and hp in (2, 3)
                    p_A = PA.tile([128, 1536], BF16, tag="p_sb", name="p_A")
                    if offl:
                        p_B = PA.tile([128, 1536], mybir.dt.int16,
                                      tag="p_sb", name="p_B16")
                    else:
                        p_B = PA.tile([128, 1536], BF16, tag="p_sb", name="p_B")
                    svA = sc_A.rearrange("p (m q) -> p m q", q=slot)[:, 0:nb, 0:wb]
                    svB = sc_B.rearrange("p (m q) -> p m q", q=slot)[:, 0:nb, 0:wb]
                    pvA = p_A[:, 0:nb * wb].rearrange("p (m q) -> p m q", m=nb)
                    if offl:
                        pv16 = p_B[:, 0:nb * wb].rearrange(
                            "p (m q) -> p m q", m=nb)
                        pvB = p_B[:, 0:nb * wb].bitcast(BF16).rearrange(
                            "p (m q) -> p m q", m=nb)
                    else:
                        pvB = p_B[:, 0:nb * wb].rearrange("p (m q) -> p m q", m=nb)
                    for mi in range(nb):
                        m = 8 * g + ml0 + mi
                        r, j = m // 4, m % 4
                        nc.tensor.matmul(
                            svA[:, mi, :],
                            ktg[0:DH, r, hp, j * 128:(j + 1) * 128],
                            qt_sb[0:DH, hp, woff:SL],
                            start=True, stop=True)
                        nc.tensor.matmul(
                            svB[:, mi, :],
                            ktg[DH:128, r, hp, j * 128:(j + 1) * 128],
                            qt_sb[DH:128, hp, woff:SL],
                            start=True, stop=True)
                    nc.scalar.activation(
                        pvA, svA, mybir.ActivationFunctionType.Exp, scale=0.125)
                    if offl:
                        # exp(s*0.125) ~ bf16-bitcast(int16(s*23.083 + 15917.5))
                        nc.vector.tensor_scalar(
                            pv16, svB, 0.125 * 128.0 / math.log(2.0),
                            127.0 * 128.0 - 338.5,
                            mybir.AluOpType.mult, mybir.AluOpType.add)
                    else:
                        nc.scalar.activation(
                            pvB, svB, mybir.ActivationFunctionType.Exp,
                            scale=0.125)
                    mks = mk_sb[:, ml0:ml0 + nb, 16 * ml0:128]
                    nc.vector.tensor_mul(pvA[:, :, 0:mw], pvA[:, :, 0:mw], mks)
                    nc.vector.tensor_mul(pvB[:, :, 0:mw], pvB[:, :, 0:mw], mks)

                    def av(hp=hp, g=g, ml0=ml0, nb=nb, woff=woff,
                           pvA=pvA, pvB=pvB, o_A=o_A, o_B=o_B):
                        for mi in range(nb):
                            m = 8 * g + ml0 + mi
                            r, j = m // 4, m % 4
                            nc.tensor.matmul(
                                o_A[:, woff:SL], vog[:, r, j, 2 * hp, :],
                                pvA[:, mi, :],
                                start=(m == 0), stop=(m == NM - 1))
                            nc.tensor.matmul(
                                o_B[:, woff:SL], vog[:, r, j, 2 * hp + 1, :],
                                pvB[:, mi, :],
                                start=(m == 0), stop=(m == NM - 1))

                    pend.append((av, None))
                    flush(LAG)
                    ml0 += nb

            def norm(hp=hp, o_A=o_A, o_B=o_B):
                # att = o[0:64] * (1/denom) broadcast over partitions
                for po, o_ps in ((0, o_A), (64, o_B)):
                    rd = PB.tile([1, SL], F32, tag="rd")
                    nc.vector.reciprocal(rd, o_ps[DH:DH + 1, :])
                    rd_d = PRD.tile([SL], F32, tag="rd_d")
                    nc.sync.dma_start(out=rd_d[None, :], in_=rd)
                    b_sb = PB.tile([DH, SL], F32, tag="b_sb")
                    nc.sync.dma_start(
                        out=b_sb,
                        in_=bass.AP(tensor=rd_d.tensor, offset=rd_d.offset,
                                    ap=[[0, DH], [1, SL]]))
                    nc.vector.tensor_mul(
                        att_sb[po:po + DH, hp, :], o_ps[0:DH, :], b_sb)

            # attach the normalize to the last AV batch of this head pair
            if pend:
                av_last, _ = pend[-1]
                pend[-1] = (av_last, norm)
            else:
                norm()
        flush(0)

        PRD.release()
        PB.release()
        PA.release()
        PO.release()

        # ---- output projection ----
        PS.release()
        PY = tc.alloc_tile_pool(name="y_ps", bufs=2, space="PSUM")
        PYW = tc.alloc_tile_pool(name="y_work", bufs=2)
        for j in range(NJ):
            y_ps = PY.tile([128, D], F32, tag="y_ps")
            for dc in range(NDC):
                lt = att_sb[:, dc, j * 128:(j + 1) * 128]
                nc.tensor.matmul(y_ps[:, 0:512], lt, wo_sb[:, dc, 0:512],
                                 start=(dc == 0), stop=(dc == NDC - 1))
                nc.tensor.matmul(y_ps[:, 512:768], lt, wo_sb[:, dc, 512:768],
                                 start=(dc == 0), stop=(dc == NDC - 1))
            y_sb = PYW.tile([128, D], BF16, tag="y_sb")
            nc.vector.tensor_copy(y_sb, y_ps)
            nc.sync.dma_start(out=y_d[j * 128:(j + 1) * 128, :], in_=y_sb)
        PYW.release()
        PY.release()
        PD.release()
        P1.release()

    nc.compile()
    return nc


_NC_CACHE = None


def _get_nc():
    global _NC_CACHE
    if _NC_CACHE is None:
        _NC_CACHE = build_nc()
    return _NC_CACHE


def _col_perm():
    """W_q/W_k column permutation: per head, de-interleave rope pairs into
    [x0(32) | x1(32)] blocks so the rotation is a unit-stride vector op."""
    return np.concatenate(
        [h * 64 + np.concatenate([np.arange(0, 64, 2), np.arange(1, 64, 2)])
         for h in range(H)])


def _pmajor(w):
    """[D, D] -> [128, NDC*D] partition-major contiguous."""
    return np.ascontiguousarray(
        w.reshape(NDC, 128, D).transpose(1, 0, 2).reshape(128, NDC * D))


def make_in_maps(x, rope_freqs, W_q, W_k, W_v, W_o):
    x2 = np.asarray(x, np.float32).reshape(S, D)
    cos = np.cos(np.asarray(rope_freqs, np.float32)).astype(BF)
    sin = np.sin(np.asarray(rope_freqs, np.float32)).astype(BF)
    perm = _col_perm()
    wq_p = _pmajor(np.asarray(W_q, np.float32)[:, perm].astype(BF))
    wk_p = _pmajor(np.asarray(W_k, np.float32)[:, perm].astype(BF))
    wv_b = _pmajor(np.asarray(W_v, np.float32).astype(BF))
    wo_b = _pmajor(np.asarray(W_o, np.float32).astype(BF))
    xT = x2.T.astype(BF)                       # [D, S]
    xq_all = xT.reshape(NDC, 128, SL, NC)      # [:, :, s, c] = strided q rows
    xkv_all = xT.reshape(NDC, 128, NC, SL)

    # cos/sin: [S, 32] -> [128, NJ*32] (heads broadcast on device, stride-0)
    def rope_tab(tab, rows):
        tt = tab[rows].reshape(NJ, 128, 32)
        return np.ascontiguousarray(tt.transpose(1, 0, 2)).reshape(128, NJ * 32)

    kr = np.arange(128)[:, None, None]
    ml = np.arange(8)[None, :, None]
    col = np.arange(128)[None, None, :]
    in_maps = []
    for c in range(NC):
        xq_t = np.ascontiguousarray(
            xq_all[:, :, :, c].transpose(1, 0, 2)).reshape(128, NDC * SL)
        xkv_t = np.ascontiguousarray(
            xkv_all[:, :, c, :].transpose(1, 0, 2)).reshape(128, NDC * SL)
        qrows = np.arange(SL) * NC + c
        krows = np.arange(SL * c, SL * (c + 1))
        mk = (128 * ml + kr <= 8 * col + c).astype(BF).reshape(128, 8 * 128)
        in_maps.append({
            "xq_t": xq_t, "xkv_t": xkv_t,
            "wq": wq_p, "wk": wk_p, "wv": wv_b, "wo": wo_b,
            "cosq": rope_tab(cos, qrows), "sinq": rope_tab(sin, qrows),
            "cosk": rope_tab(cos, krows), "sink": rope_tab(sin, krows),
            "mask8": mk,
        })
    return in_maps


_EXEC_CACHE = None


def _get_exec():
    """Cached jitted PJRT executable for the compiled Bass module (the stock
    run path re-traces and re-compiles the XLA wrapper on every call)."""
    global _EXEC_CACHE
    if _EXEC_CACHE is not None:
        return _EXEC_CACHE
    import jax
    from jax.sharding import Mesh, PartitionSpec
    from jax.experimental.shard_map import shard_map
    from concourse import bass2jax

    nc = _get_nc()
    bass2jax.install_neuronx_cc_hook()
    pname = nc.partition_id_tensor.name if nc.partition_id_tensor else None
    in_names, out_names, out_avals, zero_outs = [], [], [], []
    for alloc in nc.m.functions[0].allocations:
        if not isinstance(alloc, bass2jax.mybir.MemoryLocationSet):
            continue
        name = alloc.memorylocations[0].name
        if alloc.kind == "ExternalInput":
            if name != pname:
                in_names.append(name)
        elif alloc.kind == "ExternalOutput":
            shape = tuple(alloc.tensor_shape)
            dtype = bass2jax.mybir.dt.np(alloc.dtype)
            out_avals.append(jax.core.ShapedArray(shape, dtype))
            out_names.append(name)
            zero_outs.append(
                np.zeros((NC * shape[0], *shape[1:]), dtype))
    n_params = len(in_names)
    all_names = in_names + out_names
    if pname is not None:
        all_names = all_names + [pname]

    def _body(*args):
        operands = list(args)
        if pname is not None:
            operands.append(bass2jax.partition_id_tensor())
        outs = bass2jax._bass_exec_p.bind(
            *operands, out_avals=tuple(out_avals), in_names=tuple(all_names),
            out_names=tuple(out_names), lowering_input_output_aliases=(),
            sim_require_finite=True, sim_require_nnan=True, nc=nc)
        return tuple(outs)

    devices = jax.devices()[:NC]
    mesh = Mesh(np.asarray(devices), ("core",))
    specs = (PartitionSpec("core"),) * (n_params + len(out_names))
    fn = jax.jit(shard_map(_body, mesh=mesh, in_specs=specs,
                           out_specs=(PartitionSpec("core"),) * len(out_names),
                           check_rep=False))
    zeros_dev = [jax.device_put(z) for z in zero_outs]
    _EXEC_CACHE = (fn, in_names, n_params, zeros_dev)
    return _EXEC_CACHE


def kernel(x, rope_freqs, W_q, W_k, W_v, W_o):
    fn, in_names, n_params, zeros_dev = _get_exec()
    in_maps = make_in_maps(x, rope_freqs, W_q, W_k, W_v, W_o)
    concat_in = [
        np.concatenate([np.asarray(in_maps[c][nm]) for c in range(NC)], 0)
        for nm in in_names
    ]
    out_arrs = fn(*concat_in, *zeros_dev)
    y = np.asarray(out_arrs[0]).reshape(NC, SL, D)
    out = np.empty((S, D), np.float32)
    for c in range(NC):
        out[c::NC, :] = y[c].astype(np.float32)
    return out.reshape(1, S, D)


# revision 36
# speedup vs baseline: 1.0104x; 1.0104x over previous
"""Trainium2 Bass kernel: causal multi-head attention with RoPE (B=1, S=4096,
D=768, H=12) distributed over 8 NeuronCores.

Sharding strategy
-----------------
- Q rows are strided across cores (core c owns rows r = c mod 8) so causal
  work is uniform across cores (the SPMD program is identical on every core).
- K/V projections are computed on contiguous 512-row shards per core, RoPE'd
  and transposed locally, then AllGather'd so every core holds full K/V.
- Attention runs in "transposed scores" layout: S^T[k, q] = K_rope @ Q_rope^T
  so the AV matmul consumes exp(S^T) directly, and a ones-column appended to V
  yields the softmax denominators in the same accumulation.  Softmax is
  computed without max-subtraction (scores ~N(0,1)).
- All math is bf16 (fp8 q/k quantization alone costs 2.7e-2 relative error —
  over the accuracy gate — so the tensor engine runs bf16 throughout).
- AV matmuls are software-pipelined several batches behind their exp so the
  in-order PE stream never stalls on the later-arriving gathered V.
- RoPE pairs are de-interleaved by permuting W_q/W_k columns host-side so the
  rotation is a full-width unit-stride vector op.
- All DRAM inputs are partition-major contiguous so every load is one
  descriptor per partition.
"""

import math
import os as _os
import sys

import numpy as np

sys.path.insert(0, "/opt/trn_rl_repo")

import ml_dtypes

import concourse.bass as bass
import concourse.mybir as mybir
import concourse.tile as tile
from concourse import bacc
from concourse.masks import make_identity

BF = ml_dtypes.bfloat16
F32 = mybir.dt.float32
BF16 = mybir.dt.bfloat16

S, D, H, DH = 4096, 768, 12, 64
NC = 8
SL = S // NC          # 512 rows per core (both q-strided and kv-contiguous)
NJ = SL // 128        # 4 row-tiles per core
NM = S // 128         # 32 k-tiles
NDC = D // 128        # 6 contraction chunks == head pairs
H32 = H * 32          # 384

# Concurrent xbar transposes on two HWDGE queues race on real hardware
# (verified: nondeterministic corruption) — keep them on one queue.
F_T2Q = _os.environ.get("K_T2Q", "0") == "1"
F_LAG = int(_os.environ.get("K_LAG", "13"))      # AV software-pipeline depth
F_WARM = _os.environ.get("K_WARM", "1") == "1"   # PE p-state warmup
F_BC0 = _os.environ.get("K_BC0", "1") == "1"     # stride-0 cos/sin broadcast
F_PET = _os.environ.get("K_PET", "1") == "1"     # PE transposes + ACT copies
# DVE Schraudolph exp (int16 bitcast) relieves the scalar engine but puts
# the slower DVE inside the score-PSUM recycle chain: measured +4.5us. Off.
F_SCH = _os.environ.get("K_SCH", "0") == "1"


def build_nc():
    nc = bacc.Bacc(None, target_bir_lowering=False, debug=False)

    xq_t = nc.dram_tensor("xq_t", [128, NDC * SL], BF16, kind="ExternalInput")
    xkv_t = nc.dram_tensor("xkv_t", [128, NDC * SL], BF16, kind="ExternalInput")
    wq = nc.dram_tensor("wq", [128, NDC * D], BF16, kind="ExternalInput")
    wk = nc.dram_tensor("wk", [128, NDC * D], BF16, kind="ExternalInput")
    wv = nc.dram_tensor("wv", [128, NDC * D], BF16, kind="ExternalInput")
    wo = nc.dram_tensor("wo", [128, NDC * D], BF16, kind="ExternalInput")
    cosq = nc.dram_tensor("cosq", [128, NJ * 32], BF16, kind="ExternalInput")
    sinq = nc.dram_tensor("sinq", [128, NJ * 32], BF16, kind="ExternalInput")
    cosk = nc.dram_tensor("cosk", [128, NJ * 32], BF16, kind="ExternalInput")
    sink = nc.dram_tensor("sink", [128, NJ * 32], BF16, kind="ExternalInput")
    mask8 = nc.dram_tensor("mask8", [128, 8 * 128], BF16, kind="ExternalInput")
    y_d = nc.dram_tensor("y", [SL, D], BF16, kind="ExternalOutput")

    KT_N = 128 * NDC * SL             # elements of one core's k^T shard
    V_N = 128 * NJ * H * (DH + 1)

    with tile.TileContext(nc) as tc:
        # ---- persistent pool (lives to the end) ----
        P1 = tc.alloc_tile_pool(name="persist", bufs=1)
        wo_sb = P1.tile([128, NDC, D], BF16)
        mk_sb = P1.tile([128, 8, 128], BF16)
        qt_sb = P1.tile([128, NDC, SL], BF16)         # q^T (rope'd)
        att_sb = P1.tile([128, NDC, SL], BF16)        # attention out^T (normed)
        ktg = P1.tile([128, NC, NDC, SL], BF16)       # gathered k^T, r-outer
        vog = P1.tile([128, NC, NJ, H, DH + 1], BF16)  # gathered V (+ones col)

        PD = tc.alloc_tile_pool(name="dram", bufs=1, space="DRAM")
        kt_b = PD.tile([KT_N], BF16)
        v_b = PD.tile([V_N], BF16)
        kt_g = PD.tile([NC * KT_N], BF16, addr_space="Shared")
        v_g = PD.tile([NC * V_N], BF16, addr_space="Shared")

        # ---- projection + rope + transpose for one stream ----
        # r_sb column order per head: [y0(32) | y1(32)], heads in order, so
        # the per-(st, dc) [128,128] transpose lands chunk dc's two heads on
        # partitions [0:64) / [64:128) — the K=64 score-matmul layout.
        def proj_rope_t(x_sb, w_sb, cos_sb, sin_sb, dst_bf, ps_bufs=2,
                        warm=None, ident=None, cp_eng=None):
            PP = tc.alloc_tile_pool(name="proj_ps", bufs=ps_bufs, space="PSUM")
            if F_PET:
                PT = tc.alloc_tile_pool(name="tr_ps", bufs=3, space="PSUM")
            PW = tc.alloc_tile_pool(name="proj_work", bufs=2)
            if warm is not None and F_WARM:
                w_ps = PP.tile([128, 512], F32, tag="warm", bufs=1)
                for _ in range(9):
                    nc.tensor.matmul(w_ps, warm[:, 0:128], warm,
                                     start=True, stop=True)
            pend_t = []
            for st in range(NJ):
                n_ps = PP.tile([128, D], F32, tag="n_ps")
                for dc in range(NDC):
                    lt = x_sb[:, dc, st * 128:(st + 1) * 128]
                    nc.tensor.matmul(n_ps[:, 0:512], lt, w_sb[:, dc, 0:512],
                                     start=(dc == 0), stop=(dc == NDC - 1))
                    nc.tensor.matmul(n_ps[:, 512:768], lt, w_sb[:, dc, 512:768],
                                     start=(dc == 0), stop=(dc == NDC - 1))
                # previous row-tile's transposes go to the PE *after* this
                # tile's matmuls so the in-order PE never waits on the rope
                for fn_ in pend_t:
                    fn_()
                pend_t = []
                nb = PW.tile([128, H, 2, 32], BF16, tag="nb")
                nc.vector.tensor_copy(
                    nb.rearrange("p h x i -> p (h x i)"), n_ps)
                x0 = nb[:, :, 0]
                x1 = nb[:, :, 1]
                c0 = cos_sb[:, st]
                s0 = sin_sb[:, st]
                if F_BC0:
                    cs = bass.AP(tensor=c0.tensor, offset=c0.offset,
                                 ap=[list(c0.ap[0]), [0, H], [1, 32]])
                    sn = bass.AP(tensor=s0.tensor, offset=s0.offset,
                                 ap=[list(s0.ap[0]), [0, H], [1, 32]])
                else:
                    csf = PW.tile([128, H, 32], BF16, tag="csf")
                    snf = PW.tile([128, H, 32], BF16, tag="snf")
                    for h in range(H):
                        nc.vector.tensor_copy(csf[:, h], c0)
                        nc.vector.tensor_copy(snf[:, h], s0)
                    cs, sn = csf, snf
                ta = PW.tile([128, H, 32], BF16, tag="ta")
                tb = PW.tile([128, H, 32], BF16, tag="tb")
                tc2 = PW.tile([128, H, 32], BF16, tag="tc")
                td = PW.tile([128, H, 32], BF16, tag="td")
                r_sb = PW.tile([128, H, 2, 32], BF16, tag="r_sb")
                nc.vector.tensor_mul(ta, x0, cs)
                nc.vector.tensor_mul(tb, x1, sn)
                nc.vector.tensor_sub(r_sb[:, :, 0], ta, tb)
                nc.vector.tensor_mul(tc2, x0, sn)
                nc.vector.tensor_mul(td, x1, cs)
                nc.vector.tensor_add(r_sb[:, :, 1], tc2, td)
                rf = r_sb.rearrange("p h x i -> p (h x i)")
                if F_PET:
                    def tjob(rf=rf, st=st):
                        # PE transpose + copy on an idle engine (scalar for
                        # the K stream; vector for Q so the in-order scalar
                        # queue is clear when the first exp arrives)
                        for dc in range(NDC):
                            t_ps = PT.tile([128, 128], BF16, tag="t_ps")
                            nc.tensor.transpose(
                                t_ps, rf[:, dc * 128:(dc + 1) * 128], ident)
                            if cp_eng is nc.vector:
                                nc.vector.tensor_copy(
                                    dst_bf[:, dc, st * 128:(st + 1) * 128],
                                    t_ps)
                            else:
                                nc.scalar.activation(
                                    dst_bf[:, dc, st * 128:(st + 1) * 128],
                                    t_ps, mybir.ActivationFunctionType.Copy)
                    pend_t.append(tjob)
                else:
                    for dc in range(NDC):
                        eng = (nc.sync if dc % 2 == 0 or not F_T2Q
                               else nc.scalar)
                        eng.dma_start(
                            out=dst_bf[:, dc, st * 128:(st + 1) * 128],
                            in_=rf[:, dc * 128:(dc + 1) * 128],
                            transpose=True)
            for fn_ in pend_t:
                fn_()
            PW.release()
            if F_PET:
                PT.release()
            PP.release()

        def v_proj(x_sb, v_w_sb, v_dst):
            PP = tc.alloc_tile_pool(name="vproj_ps", bufs=2, space="PSUM")
            for st in range(NJ):
                v_ps = PP.tile([128, D], F32, tag="v_ps")
                for dc in range(NDC):
                    lt = x_sb[:, dc, st * 128:(st + 1) * 128]
                    nc.tensor.matmul(v_ps[:, 0:512], lt, v_w_sb[:, dc, 0:512],
                                     start=(dc == 0), stop=(dc == NDC - 1))
                    nc.tensor.matmul(v_ps[:, 512:768], lt,
                                     v_w_sb[:, dc, 512:768],
                                     start=(dc == 0), stop=(dc == NDC - 1))
                nc.vector.tensor_copy(
                    v_dst[:, st, :, 0:DH],
                    v_ps.rearrange("p (h d) -> p h d", h=H))
            PP.release()

        # ---- input loads (K-path inputs first; Q/O loads deferred) ----
        P2 = tc.alloc_tile_pool(name="kv_in", bufs=1)
        wk_sb = P2.tile([128, NDC, D], BF16)
        xkv_sb = P2.tile([128, NDC, SL], BF16)
        HC, HD, HS = NDC // 2, NDC // 2 * D, NDC // 2 * SL
        nc.sync.dma_start(out=wk_sb[:, 0:HC].rearrange("p c d -> p (c d)"),
                          in_=wk[:, 0:HD])
        nc.sync.dma_start(out=xkv_sb[:, 0:HC].rearrange("p c s -> p (c s)"),
                          in_=xkv_t[:, 0:HS])
        nc.sync.dma_start(out=wk_sb[:, HC:].rearrange("p c d -> p (c d)"),
                          in_=wk[:, HD:])
        nc.sync.dma_start(out=xkv_sb[:, HC:].rearrange("p c s -> p (c s)"),
                          in_=xkv_t[:, HS:])
        ck_sb = P2.tile([128, NJ, 32], BF16)
        nc.scalar.dma_start(out=ck_sb.rearrange("p t d -> p (t d)"), in_=cosk[:, :])
        sk_sb = P2.tile([128, NJ, 32], BF16)
        nc.scalar.dma_start(out=sk_sb.rearrange("p t d -> p (t d)"), in_=sink[:, :])
        P3 = tc.alloc_tile_pool(name="q_in", bufs=1)
        cq_sb = P3.tile([128, NJ, 32], BF16)
        nc.scalar.dma_start(out=cq_sb.rearrange("p t d -> p (t d)"), in_=cosq[:, :])
        sq_sb = P3.tile([128, NJ, 32], BF16)
        nc.scalar.dma_start(out=sq_sb.rearrange("p t d -> p (t d)"), in_=sinq[:, :])
        wv_sb = P2.tile([128, NDC, D], BF16)
        nc.sync.dma_start(out=wv_sb.rearrange("p c d -> p (c d)"), in_=wv[:, :])
        wq_sb = P3.tile([128, NDC, D], BF16)
        xq_sb = P3.tile([128, NDC, SL], BF16)
        kts_bf = P2.tile([128, NDC, SL], BF16)
        vs_sb = P2.tile([128, NJ, H, DH + 1], BF16)
        nc.vector.memset(vs_sb[:, :, :, DH:DH + 1], 1.0)
        warm_sb = P2.tile([128, 512], BF16)
        nc.vector.memset(warm_sb, 0.0)
        ident = P2.tile([128, 128], BF16)
        make_identity(nc, ident)

        # ---- K shard (critical path to the AllGather) ----
        proj_rope_t(xkv_sb, wk_sb, ck_sb, sk_sb, kts_bf, warm=warm_sb,
                    ident=ident)
        nc.sync.dma_start(
            out=kt_b[:].rearrange("(p n) -> p n", p=128),
            in_=kts_bf.rearrange("p c s -> p (c s)"))
        nc.gpsimd.collective_compute(
            "AllGather", mybir.AluOpType.bypass,
            replica_groups=[list(range(NC))],
            ins=[kt_b[:]], outs=[kt_g[:]],
        )
        # deferred loads: issued only after the K-path DMAs so they don't
        # crowd the descriptor channel ahead of the first collective
        nc.scalar.dma_start(out=wq_sb.rearrange("p c d -> p (c d)"), in_=wq[:, :])
        nc.scalar.dma_start(out=xq_sb.rearrange("p c s -> p (c s)"), in_=xq_t[:, :])
        nc.scalar.dma_start(out=wo_sb.rearrange("p c d -> p (c d)"), in_=wo[:, :])
        nc.scalar.dma_start(
            out=mk_sb.rearrange("p m q -> p (m q)"), in_=mask8[:, :])

        # ---- V shard (store must land before the V AllGather slot) ----
        v_proj(xkv_sb, wv_sb, vs_sb)
        nc.sync.dma_start(
            out=v_b[:].rearrange("(p n) -> p n", p=128),
            in_=vs_sb.rearrange("p t h e -> p (t h e)"))
        nc.gpsimd.collective_compute(
            "AllGather", mybir.AluOpType.bypass,
            replica_groups=[list(range(NC))],
            ins=[v_b[:]], outs=[v_g[:]],
        )

        # ---- Q shard (overlaps the collectives) ----
        proj_rope_t(xq_sb, wq_sb, cq_sb, sq_sb, qt_sb, ident=ident)
        P3.release()
        P2.release()
        PS = tc.alloc_tile_pool(name="sc_ps", bufs=3, space="PSUM")

        # ---- load gathered K/V into SBUF caches ----
        ktg_view = kt_g.rearrange("(r p c s) -> r p c s", r=NC, p=128, c=NDC)
        nc.sync.dma_start(out=ktg[:, 0, 0], in_=ktg_view[0, :, 0])
        nc.sync.dma_start(
            out=ktg[:, 0, 1:].rearrange("p c s -> p (c s)"),
            in_=ktg_view[0, :, 1:].rearrange("p c s -> p (c s)"))
        for r in range(1, NC):
            nc.sync.dma_start(
                out=ktg[:, r].rearrange("p c s -> p (c s)"),
                in_=ktg_view[r].rearrange("p c s -> p (c s)"))
        vg_view = v_g.rearrange("(r p n) -> r p n", r=NC, p=128)
        for r in range(NC):
            nc.gpsimd.dma_start(
                out=vog[:, r].rearrange("p t h e -> p (t h e)"),
                in_=vg_view[r])

        # ---- attention ----
        PO = tc.alloc_tile_pool(name="o_ps", bufs=2, space="PSUM")
        PA = tc.alloc_tile_pool(name="att_work", bufs=32)
        PB = tc.alloc_tile_pool(name="bc_work", bufs=2)
        PRD = tc.alloc_tile_pool(name="rd_dram", bufs=2, space="DRAM")

        # AV matmuls are emitted LAG batches behind their exp so the in-order
        # PE stream never stalls on the (later-arriving) gathered V.
        LAG = F_LAG
        pend = []          # (emit_av_closure, normalize_closure_or_None)

        def flush(n):
            while len(pend) > n:
                av, fin = pend.pop(0)
                av()
                if fin is not None:
                    fin()

        for hp in range(H // 2):
            o_A = PO.tile([DH + 1, SL], F32, tag="o_ps", name="o_A")
            o_B = PO.tile([DH + 1, SL], F32, tag="o_ps", name="o_B")
            for g in range(NJ):
                qoff = g * 128
                ml0 = 0
                while ml0 < 8:
                    woff = qoff + 16 * ml0
                    wb = SL - woff
                    # slot stride: each member must stay inside one PSUM bank
                    slot = 512 if wb > 256 else (256 if wb > 128 else 128)
                    nb = min(8 - ml0, 1024 // slot)
                    mw = 128 - 16 * ml0
                    sc_A = PS.tile([128, 1024], F32, tag="sc_ps", name="sc_A")
                    sc_B = PS.tile([128, 1024], F32, tag="sc_ps", name="sc_B")
                    # head pairs 2-3 route the odd head's exp to the vector
                    # engine: bf16 Schraudolph (int16(x*A+B) bitcast to bf16)
                    # relieves the scalar engine, and the bitcast view keeps
                    # the mask multiply in 4x mode and feeds AV directly.
                    offl = F_SCH and hp in (2, 3)
                    p_A = PA.tile([128, 1024], BF16, tag="p_sb", name="p_A")
                    if offl:
                        p_B = PA.tile([128, 1024], mybir.dt.int16,
                                      tag="p_sb", name="p_B16")
                    else:
                        p_B = PA.tile([128, 1024], BF16, tag="p_sb", name="p_B")
                    svA = sc_A.rearrange("p (m q) -> p m q", q=slot)[:, 0:nb, 0:wb]
                    svB = sc_B.rearrange("p (m q) -> p m q", q=slot)[:, 0:nb, 0:wb]
                    pvA = p_A[:, 0:nb * wb].rearrange("p (m q) -> p m q", m=nb)
                    if offl:
                        pv16 = p_B[:, 0:nb * wb].rearrange(
                            "p (m q) -> p m q", m=nb)
                        pvB = p_B[:, 0:nb * wb].bitcast(BF16).rearrange(
                            "p (m q) -> p m q", m=nb)
                    else:
                        pvB = p_B[:, 0:nb * wb].rearrange("p (m q) -> p m q", m=nb)
                    for mi in range(nb):
                        m = 8 * g + ml0 + mi
                        r, j = m // 4, m % 4
                        nc.tensor.matmul(
                            svA[:, mi, :],
                            ktg[0:DH, r, hp, j * 128:(j + 1) * 128],
                            qt_sb[0:DH, hp, woff:SL],
                            start=True, stop=True)
                        nc.tensor.matmul(
                            svB[:, mi, :],
                            ktg[DH:128, r, hp, j * 128:(j + 1) * 128],
                            qt_sb[DH:128, hp, woff:SL],
                            start=True, stop=True)
                    nc.scalar.activation(
                        pvA, svA, mybir.ActivationFunctionType.Exp, scale=0.125)
                    if offl:
                        # exp(s*0.125) ~ bf16-bitcast(int16(s*23.083 + 15917.5))
                        nc.vector.tensor_scalar(
                            pv16, svB, 0.125 * 128.0 / math.log(2.0),
                            127.0 * 128.0 - 338.5,
                            mybir.AluOpType.mult, mybir.AluOpType.add)
                    else:
                        nc.scalar.activation(
                            pvB, svB, mybir.ActivationFunctionType.Exp,
                            scale=0.125)
                    mks = mk_sb[:, ml0:ml0 + nb, 16 * ml0:128]
                    nc.vector.tensor_mul(pvA[:, :, 0:mw], pvA[:, :, 0:mw], mks)
                    nc.vector.tensor_mul(pvB[:, :, 0:mw], pvB[:, :, 0:mw], mks)

                    def av(hp=hp, g=g, ml0=ml0, nb=nb, woff=woff,
                           pvA=pvA, pvB=pvB, o_A=o_A, o_B=o_B):
                        for mi in range(nb):
                            m = 8 * g + ml0 + mi
                            r, j = m // 4, m % 4
                            nc.tensor.matmul(
                                o_A[:, woff:SL], vog[:, r, j, 2 * hp, :],
                                pvA[:, mi, :],
                                start=(m == 0), stop=(m == NM - 1))
                            nc.tensor.matmul(
                                o_B[:, woff:SL], vog[:, r, j, 2 * hp + 1, :],
                                pvB[:, mi, :],
                                start=(m == 0), stop=(m == NM - 1))

                    pend.append((av, None))
                    flush(LAG)
                    ml0 += nb

            def norm(hp=hp, o_A=o_A, o_B=o_B):
                # att = o[0:64] * (1/denom) broadcast over partitions
                for po, o_ps in ((0, o_A), (64, o_B)):
                    rd = PB.tile([1, SL], F32, tag="rd")
                    nc.vector.reciprocal(rd, o_ps[DH:DH + 1, :])
                    rd_d = PRD.tile([SL], F32, tag="rd_d")
                    nc.sync.dma_start(out=rd_d[None, :], in_=rd)
                    b_sb = PB.tile([DH, SL], F32, tag="b_sb")
                    nc.sync.dma_start(
                        out=b_sb,
                        in_=bass.AP(tensor=rd_d.tensor, offset=rd_d.offset,
                                    ap=[[0, DH], [1, SL]]))
                    nc.vector.tensor_mul(
                        att_sb[po:po + DH, hp, :], o_ps[0:DH, :], b_sb)

            # attach the normalize to the last AV batch of this head pair
            if pend:
                av_last, _ = pend[-1]
                pend[-1] = (av_last, norm)
            else:
                norm()
        flush(0)

        PRD.release()
        PB.release()
        PA.release()
        PO.release()

        # ---- output projection ----
        PS.release()
        PY = tc.alloc_tile_pool(name="y_ps", bufs=2, space="PSUM")
        PYW = tc.alloc_tile_pool(name="y_work", bufs=2)
        for j in range(NJ):
            y_ps = PY.tile([128, D], F32, tag="y_ps")
            for dc in range(NDC):
                lt = att_sb[:, dc, j * 128:(j + 1) * 128]
                nc.tensor.matmul(y_ps[:, 0:512], lt, wo_sb[:, dc, 0:512],
                                 start=(dc == 0), stop=(dc == NDC - 1))
                nc.tensor.matmul(y_ps[:, 512:768], lt, wo_sb[:, dc, 512:768],
                                 start=(dc == 0), stop=(dc == NDC - 1))
            y_sb = PYW.tile([128, D], BF16, tag="y_sb")
            nc.vector.tensor_copy(y_sb, y_ps)
            nc.sync.dma_start(out=y_d[j * 128:(j + 1) * 128, :], in_=y_sb)
        PYW.release()
        PY.release()
        PD.release()
        P1.release()

    nc.compile()
    return nc


_NC_CACHE = None


def _get_nc():
    global _NC_CACHE
    if _NC_CACHE is None:
        _NC_CACHE = build_nc()
    return _NC_CACHE


def _col_perm():
    """W_q/W_k column permutation: per head, de-interleave rope pairs into
    [x0(32) | x1(32)] blocks so the rotation is a unit-stride vector op."""
    return np.concatenate(
        [h * 64 + np.concatenate([np.arange(0, 64, 2), np.arange(1, 64, 2)])
         for h in range(H)])


def _pmajor(w):
    """[D, D] -> [128, NDC*D] partition-major contiguous."""
    return np.ascontiguousarray(
        w.reshape(NDC, 128, D).transpose(1, 0, 2).reshape(128, NDC * D))


def make_in_maps(x, rope_freqs, W_q, W_k, W_v, W_o):
    x2 = np.asarray(x, np.float32).reshape(S, D)
    cos = np.cos(np.asarray(rope_freqs, np.float32)).astype(BF)
    sin = np.sin(np.asarray(rope_freqs, np.float32)).astype(BF)
    perm = _col_perm()
    wq_p = _pmajor(np.asarray(W_q, np.float32)[:, perm].astype(BF))
    wk_p = _pmajor(np.asarray(W_k, np.float32)[:, perm].astype(BF))
    wv_b = _pmajor(np.asarray(W_v, np.float32).astype(BF))
    wo_b = _pmajor(np.asarray(W_o, np.float32).astype(BF))
    xT = x2.T.astype(BF)                       # [D, S]
    xq_all = xT.reshape(NDC, 128, SL, NC)      # [:, :, s, c] = strided q rows
    xkv_all = xT.reshape(NDC, 128, NC, SL)

    # cos/sin: [S, 32] -> [128, NJ*32] (heads broadcast on device, stride-0)
    def rope_tab(tab, rows):
        tt = tab[rows].reshape(NJ, 128, 32)
        return np.ascontiguousarray(tt.transpose(1, 0, 2)).reshape(128, NJ * 32)

    kr = np.arange(128)[:, None, None]
    ml = np.arange(8)[None, :, None]
    col = np.arange(128)[None, None, :]
    in_maps = []
    for c in range(NC):
        xq_t = np.ascontiguousarray(
            xq_all[:, :, :, c].transpose(1, 0, 2)).reshape(128, NDC * SL)
        xkv_t = np.ascontiguousarray(
            xkv_all[:, :, c, :].transpose(1, 0, 2)).reshape(128, NDC * SL)
        qrows = np.arange(SL) * NC + c
        krows = np.arange(SL * c, SL * (c + 1))
        mk = (128 * ml + kr <= 8 * col + c).astype(BF).reshape(128, 8 * 128)
        in_maps.append({
            "xq_t": xq_t, "xkv_t": xkv_t,
            "wq": wq_p, "wk": wk_p, "wv": wv_b, "wo": wo_b,
            "cosq": rope_tab(cos, qrows), "sinq": rope_tab(sin, qrows),
            "cosk": rope_tab(cos, krows), "sink": rope_tab(sin, krows),
            "mask8": mk,
        })
    return in_maps


_EXEC_CACHE = None


def _get_exec():
    """Cached jitted PJRT executable for the compiled Bass module (the stock
    run path re-traces and re-compiles the XLA wrapper on every call)."""
    global _EXEC_CACHE
    if _EXEC_CACHE is not None:
        return _EXEC_CACHE
    import jax
    from jax.sharding import Mesh, PartitionSpec
    from jax.experimental.shard_map import shard_map
    from concourse import bass2jax

    nc = _get_nc()
    bass2jax.install_neuronx_cc_hook()
    pname = nc.partition_id_tensor.name if nc.partition_id_tensor else None
    in_names, out_names, out_avals, zero_outs = [], [], [], []
    for alloc in nc.m.functions[0].allocations:
        if not isinstance(alloc, bass2jax.mybir.MemoryLocationSet):
            continue
        name = alloc.memorylocations[0].name
        if alloc.kind == "ExternalInput":
            if name != pname:
                in_names.append(name)
        elif alloc.kind == "ExternalOutput":
            shape = tuple(alloc.tensor_shape)
            dtype = bass2jax.mybir.dt.np(alloc.dtype)
            out_avals.append(jax.core.ShapedArray(shape, dtype))
            out_names.append(name)
            zero_outs.append(
                np.zeros((NC * shape[0], *shape[1:]), dtype))
    n_params = len(in_names)
    all_names = in_names + out_names
    if pname is not None:
        all_names = all_names + [pname]

    def _body(*args):
        operands = list(args)
        if pname is not None:
            operands.append(bass2jax.partition_id_tensor())
        outs = bass2jax._bass_exec_p.bind(
            *operands, out_avals=tuple(out_avals), in_names=tuple(all_names),
            out_names=tuple(out_names), lowering_input_output_aliases=(),
            sim_require_finite=True, sim_require_nnan=True, nc=nc)
        return tuple(outs)

    devices = jax.devices()[:NC]
    mesh = Mesh(np.asarray(devices), ("core",))
    specs = (PartitionSpec("core"),) * (n_params + len(out_names))
    fn = jax.jit(shard_map(_body, mesh=mesh, in_specs=specs,
                           out_specs=(PartitionSpec("core"),) * len(out_names),
                           check_rep=False))
    zeros_dev = [jax.device_put(z) for z in zero_outs]
    _EXEC_CACHE = (fn, in_names, n_params, zeros_dev)
    return _EXEC_CACHE


def kernel(x, rope_freqs, W_q, W_k, W_v, W_o):
    fn, in_names, n_params, zeros_dev = _get_exec()
    in_maps = make_in_maps(x, rope_freqs, W_q, W_k, W_v, W_o)
    concat_in = [
        np.concatenate([np.asarray(in_maps[c][nm]) for c in range(NC)], 0)
        for nm in in_names
    ]
    out_arrs = fn(*concat_in, *zeros_dev)
    y = np.asarray(out_arrs[0]).reshape(NC, SL, D)
    out = np.empty((S, D), np.float32)
    for c in range(NC):
        out[c::NC, :] = y[c].astype(np.float32)
    return out.reshape(1, S, D)


# revision 37
# speedup vs baseline: 1.0152x; 1.0048x over previous
"""Trainium2 Bass kernel: causal multi-head attention with RoPE (B=1, S=4096,
D=768, H=12) distributed over 8 NeuronCores.

Sharding strategy
-----------------
- Q rows are strided across cores (core c owns rows r = c mod 8) so causal
  work is uniform across cores (the SPMD program is identical on every core).
- K/V projections are computed on contiguous 512-row shards per core, RoPE'd
  and transposed locally, then AllGather'd so every core holds full K/V.
- Attention runs in "transposed scores" layout: S^T[k, q] = K_rope @ Q_rope^T
  so the AV matmul consumes exp(S^T) directly, and a ones-column appended to V
  yields the softmax denominators in the same accumulation.  Softmax is
  computed without max-subtraction (scores ~N(0,1)).
- All math is bf16 (fp8 q/k quantization alone costs 2.7e-2 relative error —
  over the accuracy gate — so the tensor engine runs bf16 throughout).
- AV matmuls are software-pipelined several batches behind their exp so the
  in-order PE stream never stalls on the later-arriving gathered V.
- RoPE pairs are de-interleaved by permuting W_q/W_k columns host-side so the
  rotation is a full-width unit-stride vector op.
- All DRAM inputs are partition-major contiguous so every load is one
  descriptor per partition.
"""

import math
import os as _os
import sys

import numpy as np

sys.path.insert(0, "/opt/trn_rl_repo")

import ml_dtypes

import concourse.bass as bass
import concourse.mybir as mybir
import concourse.tile as tile
from concourse import bacc
from concourse.masks import make_identity

BF = ml_dtypes.bfloat16
F32 = mybir.dt.float32
BF16 = mybir.dt.bfloat16

S, D, H, DH = 4096, 768, 12, 64
NC = 8
SL = S // NC          # 512 rows per core (both q-strided and kv-contiguous)
NJ = SL // 128        # 4 row-tiles per core
NM = S // 128         # 32 k-tiles
NDC = D // 128        # 6 contraction chunks == head pairs
H32 = H * 32          # 384

# Concurrent xbar transposes on two HWDGE queues race on real hardware
# (verified: nondeterministic corruption) — keep them on one queue.
F_T2Q = _os.environ.get("K_T2Q", "0") == "1"
F_LAG = int(_os.environ.get("K_LAG", "13"))      # AV software-pipeline depth
F_WARM = _os.environ.get("K_WARM", "1") == "1"   # PE p-state warmup
F_BC0 = _os.environ.get("K_BC0", "1") == "1"     # stride-0 cos/sin broadcast
F_PET = _os.environ.get("K_PET", "1") == "1"     # PE transposes + ACT copies
# DVE Schraudolph exp (int16 bitcast) relieves the scalar engine but puts
# the slower DVE inside the score-PSUM recycle chain: measured +4.5us. Off.
F_SCH = _os.environ.get("K_SCH", "0") == "1"


def build_nc():
    nc = bacc.Bacc(None, target_bir_lowering=False, debug=False)

    xq_t = nc.dram_tensor("xq_t", [128, NDC * SL], BF16, kind="ExternalInput")
    xkv_t = nc.dram_tensor("xkv_t", [128, NDC * SL], BF16, kind="ExternalInput")
    wq = nc.dram_tensor("wq", [128, NDC * D], BF16, kind="ExternalInput")
    wk = nc.dram_tensor("wk", [128, NDC * D], BF16, kind="ExternalInput")
    wv = nc.dram_tensor("wv", [128, NDC * D], BF16, kind="ExternalInput")
    wo = nc.dram_tensor("wo", [128, NDC * D], BF16, kind="ExternalInput")
    cosq = nc.dram_tensor("cosq", [128, NJ * 32], BF16, kind="ExternalInput")
    sinq = nc.dram_tensor("sinq", [128, NJ * 32], BF16, kind="ExternalInput")
    cosk = nc.dram_tensor("cosk", [128, NJ * 32], BF16, kind="ExternalInput")
    sink = nc.dram_tensor("sink", [128, NJ * 32], BF16, kind="ExternalInput")
    mask8 = nc.dram_tensor("mask8", [128, 8 * 128], BF16, kind="ExternalInput")
    y_d = nc.dram_tensor("y", [SL, D], BF16, kind="ExternalOutput")

    KT_N = 128 * NDC * SL             # elements of one core's k^T shard
    V_N = 128 * NJ * H * (DH + 1)

    with tile.TileContext(nc) as tc:
        # ---- persistent pool (lives to the end) ----
        P1 = tc.alloc_tile_pool(name="persist", bufs=1)
        wo_sb = P1.tile([128, NDC, D], BF16)
        mk_sb = P1.tile([128, 8, 128], BF16)
        qt_sb = P1.tile([128, NDC, SL], BF16)         # q^T (rope'd)
        att_sb = P1.tile([128, NDC, SL], BF16)        # attention out^T (normed)
        ktg = P1.tile([128, NC, NDC, SL], BF16)       # gathered k^T, r-outer
        vog = P1.tile([128, NC, NJ, H, DH + 1], BF16)  # gathered V (+ones col)

        PD = tc.alloc_tile_pool(name="dram", bufs=1, space="DRAM")
        kt_b = PD.tile([KT_N], BF16)
        v_b = PD.tile([V_N], BF16)
        kt_g = PD.tile([NC * KT_N], BF16, addr_space="Shared")
        v_g = PD.tile([NC * V_N], BF16, addr_space="Shared")

        # ---- projection + rope + transpose for one stream ----
        # r_sb column order per head: [y0(32) | y1(32)], heads in order, so
        # the per-(st, dc) [128,128] transpose lands chunk dc's two heads on
        # partitions [0:64) / [64:128) — the K=64 score-matmul layout.
        def proj_rope_t(x_sb, w_sb, cos_sb, sin_sb, dst_bf, ps_bufs=2,
                        warm=None, ident=None, cp_eng=None):
            PP = tc.alloc_tile_pool(name="proj_ps", bufs=ps_bufs, space="PSUM")
            if F_PET:
                PT = tc.alloc_tile_pool(name="tr_ps", bufs=3, space="PSUM")
            PW = tc.alloc_tile_pool(name="proj_work", bufs=2)
            if warm is not None and F_WARM:
                w_ps = PP.tile([128, 512], F32, tag="warm", bufs=1)
                for _ in range(9):
                    nc.tensor.matmul(w_ps, warm[:, 0:128], warm,
                                     start=True, stop=True)
            pend_t = []
            for st in range(NJ):
                n_ps = PP.tile([128, D], F32, tag="n_ps")
                for dc in range(NDC):
                    lt = x_sb[:, dc, st * 128:(st + 1) * 128]
                    nc.tensor.matmul(n_ps[:, 0:512], lt, w_sb[:, dc, 0:512],
                                     start=(dc == 0), stop=(dc == NDC - 1))
                    nc.tensor.matmul(n_ps[:, 512:768], lt, w_sb[:, dc, 512:768],
                                     start=(dc == 0), stop=(dc == NDC - 1))
                # previous row-tile's transposes go to the PE *after* this
                # tile's matmuls so the in-order PE never waits on the rope
                for fn_ in pend_t:
                    fn_()
                pend_t = []
                nb = PW.tile([128, H, 2, 32], BF16, tag="nb")
                nc.vector.tensor_copy(
                    nb.rearrange("p h x i -> p (h x i)"), n_ps)
                x0 = nb[:, :, 0]
                x1 = nb[:, :, 1]
                c0 = cos_sb[:, st]
                s0 = sin_sb[:, st]
                if F_BC0:
                    cs = bass.AP(tensor=c0.tensor, offset=c0.offset,
                                 ap=[list(c0.ap[0]), [0, H], [1, 32]])
                    sn = bass.AP(tensor=s0.tensor, offset=s0.offset,
                                 ap=[list(s0.ap[0]), [0, H], [1, 32]])
                else:
                    csf = PW.tile([128, H, 32], BF16, tag="csf")
                    snf = PW.tile([128, H, 32], BF16, tag="snf")
                    for h in range(H):
                        nc.vector.tensor_copy(csf[:, h], c0)
                        nc.vector.tensor_copy(snf[:, h], s0)
                    cs, sn = csf, snf
                ta = PW.tile([128, H, 32], BF16, tag="ta")
                tb = PW.tile([128, H, 32], BF16, tag="tb")
                tc2 = PW.tile([128, H, 32], BF16, tag="tc")
                td = PW.tile([128, H, 32], BF16, tag="td")
                r_sb = PW.tile([128, H, 2, 32], BF16, tag="r_sb")
                nc.vector.tensor_mul(ta, x0, cs)
                nc.vector.tensor_mul(tb, x1, sn)
                nc.vector.tensor_sub(r_sb[:, :, 0], ta, tb)
                nc.vector.tensor_mul(tc2, x0, sn)
                nc.vector.tensor_mul(td, x1, cs)
                nc.vector.tensor_add(r_sb[:, :, 1], tc2, td)
                rf = r_sb.rearrange("p h x i -> p (h x i)")
                if F_PET:
                    def tjob(rf=rf, st=st):
                        # PE transpose + copy on an idle engine (scalar for
                        # the K stream; vector for Q so the in-order scalar
                        # queue is clear when the first exp arrives)
                        for dc in range(NDC):
                            t_ps = PT.tile([128, 128], BF16, tag="t_ps")
                            nc.tensor.transpose(
                                t_ps, rf[:, dc * 128:(dc + 1) * 128], ident)
                            if cp_eng is nc.vector:
                                nc.vector.tensor_copy(
                                    dst_bf[:, dc, st * 128:(st + 1) * 128],
                                    t_ps)
                            else:
                                nc.scalar.activation(
                                    dst_bf[:, dc, st * 128:(st + 1) * 128],
                                    t_ps, mybir.ActivationFunctionType.Copy)
                    pend_t.append(tjob)
                else:
                    for dc in range(NDC):
                        eng = (nc.sync if dc % 2 == 0 or not F_T2Q
                               else nc.scalar)
                        eng.dma_start(
                            out=dst_bf[:, dc, st * 128:(st + 1) * 128],
                            in_=rf[:, dc * 128:(dc + 1) * 128],
                            transpose=True)
            for fn_ in pend_t:
                fn_()
            PW.release()
            if F_PET:
                PT.release()
            PP.release()

        def v_proj(x_sb, v_w_sb, v_dst):
            PP = tc.alloc_tile_pool(name="vproj_ps", bufs=2, space="PSUM")
            for st in range(NJ):
                v_ps = PP.tile([128, D], F32, tag="v_ps")
                for dc in range(NDC):
                    lt = x_sb[:, dc, st * 128:(st + 1) * 128]
                    nc.tensor.matmul(v_ps[:, 0:512], lt, v_w_sb[:, dc, 0:512],
                                     start=(dc == 0), stop=(dc == NDC - 1))
                    nc.tensor.matmul(v_ps[:, 512:768], lt,
                                     v_w_sb[:, dc, 512:768],
                                     start=(dc == 0), stop=(dc == NDC - 1))
                nc.vector.tensor_copy(
                    v_dst[:, st, :, 0:DH],
                    v_ps.rearrange("p (h d) -> p h d", h=H))
            PP.release()

        # ---- input loads (K-path inputs first; Q/O loads deferred) ----
        P2 = tc.alloc_tile_pool(name="kv_in", bufs=1)
        wk_sb = P2.tile([128, NDC, D], BF16)
        xkv_sb = P2.tile([128, NDC, SL], BF16)
        HC, HD, HS = NDC // 2, NDC // 2 * D, NDC // 2 * SL
        nc.sync.dma_start(out=wk_sb[:, 0:HC].rearrange("p c d -> p (c d)"),
                          in_=wk[:, 0:HD])
        nc.sync.dma_start(out=xkv_sb[:, 0:HC].rearrange("p c s -> p (c s)"),
                          in_=xkv_t[:, 0:HS])
        nc.sync.dma_start(out=wk_sb[:, HC:].rearrange("p c d -> p (c d)"),
                          in_=wk[:, HD:])
        nc.sync.dma_start(out=xkv_sb[:, HC:].rearrange("p c s -> p (c s)"),
                          in_=xkv_t[:, HS:])
        ck_sb = P2.tile([128, NJ, 32], BF16)
        nc.scalar.dma_start(out=ck_sb.rearrange("p t d -> p (t d)"), in_=cosk[:, :])
        sk_sb = P2.tile([128, NJ, 32], BF16)
        nc.scalar.dma_start(out=sk_sb.rearrange("p t d -> p (t d)"), in_=sink[:, :])
        P3 = tc.alloc_tile_pool(name="q_in", bufs=1)
        cq_sb = P3.tile([128, NJ, 32], BF16)
        nc.scalar.dma_start(out=cq_sb.rearrange("p t d -> p (t d)"), in_=cosq[:, :])
        sq_sb = P3.tile([128, NJ, 32], BF16)
        nc.scalar.dma_start(out=sq_sb.rearrange("p t d -> p (t d)"), in_=sinq[:, :])
        wv_sb = P2.tile([128, NDC, D], BF16)
        nc.sync.dma_start(out=wv_sb.rearrange("p c d -> p (c d)"), in_=wv[:, :])
        wq_sb = P3.tile([128, NDC, D], BF16)
        xq_sb = P3.tile([128, NDC, SL], BF16)
        kts_bf = P2.tile([128, NDC, SL], BF16)
        vs_sb = P2.tile([128, NJ, H, DH + 1], BF16)
        nc.vector.memset(vs_sb[:, :, :, DH:DH + 1], 1.0)
        warm_sb = P2.tile([128, 512], BF16)
        nc.vector.memset(warm_sb, 0.0)
        ident = P2.tile([128, 128], BF16)
        make_identity(nc, ident)

        # ---- K shard (critical path to the AllGather) ----
        proj_rope_t(xkv_sb, wk_sb, ck_sb, sk_sb, kts_bf, warm=warm_sb,
                    ident=ident)
        kbv = kt_b[:].rearrange("(p c s) -> p c s", p=128, c=NDC)
        nc.sync.dma_start(
            out=kbv[:, 0:3].rearrange("p c s -> p (c s)"),
            in_=kts_bf[:, 0:3].rearrange("p c s -> p (c s)"))
        nc.sync.dma_start(
            out=kbv[:, 3:].rearrange("p c s -> p (c s)"),
            in_=kts_bf[:, 3:].rearrange("p c s -> p (c s)"))
        nc.gpsimd.collective_compute(
            "AllGather", mybir.AluOpType.bypass,
            replica_groups=[list(range(NC))],
            ins=[kt_b[:]], outs=[kt_g[:]],
        )
        # deferred loads: issued only after the K-path DMAs so they don't
        # crowd the descriptor channel ahead of the first collective
        nc.scalar.dma_start(out=wq_sb.rearrange("p c d -> p (c d)"), in_=wq[:, :])
        nc.scalar.dma_start(out=xq_sb.rearrange("p c s -> p (c s)"), in_=xq_t[:, :])
        nc.scalar.dma_start(out=wo_sb.rearrange("p c d -> p (c d)"), in_=wo[:, :])
        nc.scalar.dma_start(
            out=mk_sb.rearrange("p m q -> p (m q)"), in_=mask8[:, :])

        # ---- V shard (store must land before the V AllGather slot) ----
        v_proj(xkv_sb, wv_sb, vs_sb)
        nc.sync.dma_start(
            out=v_b[:].rearrange("(p n) -> p n", p=128),
            in_=vs_sb.rearrange("p t h e -> p (t h e)"))
        nc.gpsimd.collective_compute(
            "AllGather", mybir.AluOpType.bypass,
            replica_groups=[list(range(NC))],
            ins=[v_b[:]], outs=[v_g[:]],
        )

        # ---- Q shard (overlaps the collectives) ----
        proj_rope_t(xq_sb, wq_sb, cq_sb, sq_sb, qt_sb, ident=ident)
        P3.release()
        P2.release()
        PS = tc.alloc_tile_pool(name="sc_ps", bufs=3, space="PSUM")

        # ---- load gathered K/V into SBUF caches ----
        ktg_view = kt_g.rearrange("(r p c s) -> r p c s", r=NC, p=128, c=NDC)
        nc.sync.dma_start(out=ktg[:, 0, 0], in_=ktg_view[0, :, 0])
        nc.sync.dma_start(
            out=ktg[:, 0, 1:].rearrange("p c s -> p (c s)"),
            in_=ktg_view[0, :, 1:].rearrange("p c s -> p (c s)"))
        for r in range(1, NC):
            nc.sync.dma_start(
                out=ktg[:, r].rearrange("p c s -> p (c s)"),
                in_=ktg_view[r].rearrange("p c s -> p (c s)"))
        vg_view = v_g.rearrange("(r p n) -> r p n", r=NC, p=128)
        for r in range(NC):
            nc.gpsimd.dma_start(
                out=vog[:, r].rearrange("p t h e -> p (t h e)"),
                in_=vg_view[r])

        # ---- attention ----
        PO = tc.alloc_tile_pool(name="o_ps", bufs=2, space="PSUM")
        PA = tc.alloc_tile_pool(name="att_work", bufs=32)
        PB = tc.alloc_tile_pool(name="bc_work", bufs=2)
        PRD = tc.alloc_tile_pool(name="rd_dram", bufs=2, space="DRAM")

        # AV matmuls are emitted LAG batches behind their exp so the in-order
        # PE stream never stalls on the (later-arriving) gathered V.
        LAG = F_LAG
        pend = []          # (emit_av_closure, normalize_closure_or_None)

        def flush(n):
            while len(pend) > n:
                av, fin = pend.pop(0)
                av()
                if fin is not None:
                    fin()

        for hp in range(H // 2):
            o_A = PO.tile([DH + 1, SL], F32, tag="o_ps", name="o_A")
            o_B = PO.tile([DH + 1, SL], F32, tag="o_ps", name="o_B")
            for g in range(NJ):
                qoff = g * 128
                ml0 = 0
                while ml0 < 8:
                    woff = qoff + 16 * ml0
                    wb = SL - woff
                    # slot stride: each member must stay inside one PSUM bank
                    slot = 512 if wb > 256 else (256 if wb > 128 else 128)
                    nb = min(8 - ml0, 1024 // slot)
                    mw = 128 - 16 * ml0
                    sc_A = PS.tile([128, 1024], F32, tag="sc_ps", name="sc_A")
                    sc_B = PS.tile([128, 1024], F32, tag="sc_ps", name="sc_B")
                    # head pairs 2-3 route the odd head's exp to the vector
                    # engine: bf16 Schraudolph (int16(x*A+B) bitcast to bf16)
                    # relieves the scalar engine, and the bitcast view keeps
                    # the mask multiply in 4x mode and feeds AV directly.
                    offl = F_SCH and hp in (2, 3)
                    p_A = PA.tile([128, 1024], BF16, tag="p_sb", name="p_A")
                    if offl:
                        p_B = PA.tile([128, 1024], mybir.dt.int16,
                                      tag="p_sb", name="p_B16")
                    else:
                        p_B = PA.tile([128, 1024], BF16, tag="p_sb", name="p_B")
                    svA = sc_A.rearrange("p (m q) -> p m q", q=slot)[:, 0:nb, 0:wb]
                    svB = sc_B.rearrange("p (m q) -> p m q", q=slot)[:, 0:nb, 0:wb]
                    pvA = p_A[:, 0:nb * wb].rearrange("p (m q) -> p m q", m=nb)
                    if offl:
                        pv16 = p_B[:, 0:nb * wb].rearrange(
                            "p (m q) -> p m q", m=nb)
                        pvB = p_B[:, 0:nb * wb].bitcast(BF16).rearrange(
                            "p (m q) -> p m q", m=nb)
                    else:
                        pvB = p_B[:, 0:nb * wb].rearrange("p (m q) -> p m q", m=nb)
                    for mi in range(nb):
                        m = 8 * g + ml0 + mi
                        r, j = m // 4, m % 4
                        nc.tensor.matmul(
                            svA[:, mi, :],
                            ktg[0:DH, r, hp, j * 128:(j + 1) * 128],
                            qt_sb[0:DH, hp, woff:SL],
                            start=True, stop=True)
                        nc.tensor.matmul(
                            svB[:, mi, :],
                            ktg[DH:128, r, hp, j * 128:(j + 1) * 128],
                            qt_sb[DH:128, hp, woff:SL],
                            start=True, stop=True)
                    nc.scalar.activation(
                        pvA, svA, mybir.ActivationFunctionType.Exp, scale=0.125)
                    if offl:
                        # exp(s*0.125) ~ bf16-bitcast(int16(s*23.083 + 15917.5))
                        nc.vector.tensor_scalar(
                            pv16, svB, 0.125 * 128.0 / math.log(2.0),
                            127.0 * 128.0 - 338.5,
                            mybir.AluOpType.mult, mybir.AluOpType.add)
                    else:
                        nc.scalar.activation(
                            pvB, svB, mybir.ActivationFunctionType.Exp,
                            scale=0.125)
                    mks = mk_sb[:, ml0:ml0 + nb, 16 * ml0:128]
                    nc.vector.tensor_mul(pvA[:, :, 0:mw], pvA[:, :, 0:mw], mks)
                    nc.vector.tensor_mul(pvB[:, :, 0:mw], pvB[:, :, 0:mw], mks)

                    def av(hp=hp, g=g, ml0=ml0, nb=nb, woff=woff,
                           pvA=pvA, pvB=pvB, o_A=o_A, o_B=o_B):
                        for mi in range(nb):
                            m = 8 * g + ml0 + mi
                            r, j = m // 4, m % 4
                            nc.tensor.matmul(
                                o_A[:, woff:SL], vog[:, r, j, 2 * hp, :],
                                pvA[:, mi, :],
                                start=(m == 0), stop=(m == NM - 1))
                            nc.tensor.matmul(
                                o_B[:, woff:SL], vog[:, r, j, 2 * hp + 1, :],
                                pvB[:, mi, :],
                                start=(m == 0), stop=(m == NM - 1))

                    pend.append((av, None))
                    flush(LAG)
                    ml0 += nb

            def norm(hp=hp, o_A=o_A, o_B=o_B):
                # att = o[0:64] * (1/denom) broadcast over partitions
                for po, o_ps in ((0, o_A), (64, o_B)):
                    rd = PB.tile([1, SL], F32, tag="rd")
                    nc.vector.reciprocal(rd, o_ps[DH:DH + 1, :])
                    rd_d = PRD.tile([SL], F32, tag="rd_d")
                    nc.sync.dma_start(out=rd_d[None, :], in_=rd)
                    b_sb = PB.tile([DH, SL], F32, tag="b_sb")
                    nc.sync.dma_start(
                        out=b_sb,
                        in_=bass.AP(tensor=rd_d.tensor, offset=rd_d.offset,
                                    ap=[[0, DH], [1, SL]]))
                    nc.vector.tensor_mul(
                        att_sb[po:po + DH, hp, :], o_ps[0:DH, :], b_sb)

            # attach the normalize to the last AV batch of this head pair
            if pend:
                av_last, _ = pend[-1]
                pend[-1] = (av_last, norm)
            else:
                norm()
        flush(0)

        PRD.release()
        PB.release()
        PA.release()
        PO.release()

        # ---- output projection ----
        PS.release()
        PY = tc.alloc_tile_pool(name="y_ps", bufs=2, space="PSUM")
        PYW = tc.alloc_tile_pool(name="y_work", bufs=2)
        for j in range(NJ):
            y_ps = PY.tile([128, D], F32, tag="y_ps")
            for dc in range(NDC):
                lt = att_sb[:, dc, j * 128:(j + 1) * 128]
                nc.tensor.matmul(y_ps[:, 0:512], lt, wo_sb[:, dc, 0:512],
                                 start=(dc == 0), stop=(dc == NDC - 1))
                nc.tensor.matmul(y_ps[:, 512:768], lt, wo_sb[:, dc, 512:768],
                                 start=(dc == 0), stop=(dc == NDC - 1))
            y_sb = PYW.tile([128, D], BF16, tag="y_sb")
            nc.vector.tensor_copy(y_sb, y_ps)
            nc.sync.dma_start(out=y_d[j * 128:(j + 1) * 128, :], in_=y_sb)
        PYW.release()
        PY.release()
        PD.release()
        P1.release()

    nc.compile()
    return nc


_NC_CACHE = None


def _get_nc():
    global _NC_CACHE
    if _NC_CACHE is None:
        _NC_CACHE = build_nc()
    return _NC_CACHE


def _col_perm():
    """W_q/W_k column permutation: per head, de-interleave rope pairs into
    [x0(32) | x1(32)] blocks so the rotation is a unit-stride vector op."""
    return np.concatenate(
        [h * 64 + np.concatenate([np.arange(0, 64, 2), np.arange(1, 64, 2)])
         for h in range(H)])


def _pmajor(w):
    """[D, D] -> [128, NDC*D] partition-major contiguous."""
    return np.ascontiguousarray(
        w.reshape(NDC, 128, D).transpose(1, 0, 2).reshape(128, NDC * D))


def make_in_maps(x, rope_freqs, W_q, W_k, W_v, W_o):
    x2 = np.asarray(x, np.float32).reshape(S, D)
    cos = np.cos(np.asarray(rope_freqs, np.float32)).astype(BF)
    sin = np.sin(np.asarray(rope_freqs, np.float32)).astype(BF)
    perm = _col_perm()
    wq_p = _pmajor(np.asarray(W_q, np.float32)[:, perm].astype(BF))
    wk_p = _pmajor(np.asarray(W_k, np.float32)[:, perm].astype(BF))
    wv_b = _pmajor(np.asarray(W_v, np.float32).astype(BF))
    wo_b = _pmajor(np.asarray(W_o, np.float32).astype(BF))
    xT = x2.T.astype(BF)                       # [D, S]
    xq_all = xT.reshape(NDC, 128, SL, NC)      # [:, :, s, c] = strided q rows
    xkv_all = xT.reshape(NDC, 128, NC, SL)

    # cos/sin: [S, 32] -> [128, NJ*32] (heads broadcast on device, stride-0)
    def rope_tab(tab, rows):
        tt = tab[rows].reshape(NJ, 128, 32)
        return np.ascontiguousarray(tt.transpose(1, 0, 2)).reshape(128, NJ * 32)

    kr = np.arange(128)[:, None, None]
    ml = np.arange(8)[None, :, None]
    col = np.arange(128)[None, None, :]
    in_maps = []
    for c in range(NC):
        xq_t = np.ascontiguousarray(
            xq_all[:, :, :, c].transpose(1, 0, 2)).reshape(128, NDC * SL)
        xkv_t = np.ascontiguousarray(
            xkv_all[:, :, c, :].transpose(1, 0, 2)).reshape(128, NDC * SL)
        qrows = np.arange(SL) * NC + c
        krows = np.arange(SL * c, SL * (c + 1))
        mk = (128 * ml + kr <= 8 * col + c).astype(BF).reshape(128, 8 * 128)
        in_maps.append({
            "xq_t": xq_t, "xkv_t": xkv_t,
            "wq": wq_p, "wk": wk_p, "wv": wv_b, "wo": wo_b,
            "cosq": rope_tab(cos, qrows), "sinq": rope_tab(sin, qrows),
            "cosk": rope_tab(cos, krows), "sink": rope_tab(sin, krows),
            "mask8": mk,
        })
    return in_maps


_EXEC_CACHE = None


def _get_exec():
    """Cached jitted PJRT executable for the compiled Bass module (the stock
    run path re-traces and re-compiles the XLA wrapper on every call)."""
    global _EXEC_CACHE
    if _EXEC_CACHE is not None:
        return _EXEC_CACHE
    import jax
    from jax.sharding import Mesh, PartitionSpec
    from jax.experimental.shard_map import shard_map
    from concourse import bass2jax

    nc = _get_nc()
    bass2jax.install_neuronx_cc_hook()
    pname = nc.partition_id_tensor.name if nc.partition_id_tensor else None
    in_names, out_names, out_avals, zero_outs = [], [], [], []
    for alloc in nc.m.functions[0].allocations:
        if not isinstance(alloc, bass2jax.mybir.MemoryLocationSet):
            continue
        name = alloc.memorylocations[0].name
        if alloc.kind == "ExternalInput":
            if name != pname:
                in_names.append(name)
        elif alloc.kind == "ExternalOutput":
            shape = tuple(alloc.tensor_shape)
            dtype = bass2jax.mybir.dt.np(alloc.dtype)
            out_avals.append(jax.core.ShapedArray(shape, dtype))
            out_names.append(name)
            zero_outs.append(
                np.zeros((NC * shape[0], *shape[1:]), dtype))
    n_params = len(in_names)
    all_names = in_names + out_names
    if pname is not None:
        all_names = all_names + [pname]

    def _body(*args):
        operands = list(args)
        if pname is not None:
            operands.append(bass2jax.partition_id_tensor())
        outs = bass2jax._bass_exec_p.bind(
            *operands, out_avals=tuple(out_avals), in_names=tuple(all_names),
            out_names=tuple(out_names), lowering_input_output_aliases=(),
            sim_require_finite=True, sim_require_nnan=True, nc=nc)
        return tuple(outs)

    devices = jax.devices()[:NC]
    mesh = Mesh(np.asarray(devices), ("core",))
    specs = (PartitionSpec("core"),) * (n_params + len(out_names))
    fn = jax.jit(shard_map(_body, mesh=mesh, in_specs=specs,
                           out_specs=(PartitionSpec("core"),) * len(out_names),
                           check_rep=False))
    zeros_dev = [jax.device_put(z) for z in zero_outs]
    _EXEC_CACHE = (fn, in_names, n_params, zeros_dev)
    return _EXEC_CACHE


def kernel(x, rope_freqs, W_q, W_k, W_v, W_o):
    fn, in_names, n_params, zeros_dev = _get_exec()
    in_maps = make_in_maps(x, rope_freqs, W_q, W_k, W_v, W_o)
    concat_in = [
        np.concatenate([np.asarray(in_maps[c][nm]) for c in range(NC)], 0)
        for nm in in_names
    ]
    out_arrs = fn(*concat_in, *zeros_dev)
    y = np.asarray(out_arrs[0]).reshape(NC, SL, D)
    out = np.empty((S, D), np.float32)
    for c in range(NC):
        out[c::NC, :] = y[c].astype(np.float32)
    return out.reshape(1, S, D)
